# revision 26
# baseline (speedup 1.0000x reference)
"""DCNv2 Trainium2 Bass kernel, v3.

Problem: x[8,64,64,128], offset[8,64,64,18], modulation[8,64,64,9],
conv_kernel[3,3,128,256], conv_bias[256] -> out[8,64,64,256].
Data-parallel over batch B=8, one batch per NeuronCore.

v3 changes vs v2:
  - idx pipeline computed pix-major inside the B1 chain (reuses y0f/x0f),
    cast to i16 and rearranged to the 16-partition gather layout via two
    small scalar-queue DMAs.  Drops the whole emit_b2_chunk pipeline
    (saves ~20us serial DVE + ~16us strided loads off the critical path).
  - byp/bxp fused into one bcat const so py/px compute in one DVE op.
  - all 32 gathers enqueued immediately after idx is ready (bufs=9).
  - PSUM batching: 3 taps (one kernel-row s) share one PSUM bank; one
    ACT copy of [128,3,2,64] per (t,s) instead of 3 copies of [128,128].
  - ~250 warmup matmuls on the mask const during setup keep the PE HAM
    at 8/8 so tile 0 starts warm.
"""

import os
import sys

import numpy as np

sys.path.insert(0, "/opt/trn_rl_repo")

import concourse.bass as bass  # noqa: E402
import concourse.mybir as mybir  # noqa: E402
from concourse.tile import TileContext  # noqa: E402

F32 = mybir.dt.float32
BF16 = mybir.dt.bfloat16
FP8 = mybir.dt.float8e3
U32 = mybir.dt.uint32
I16 = mybir.dt.int16

H = W = 64
C = 128
F = 256
NK = 9
NPIX = H * W          # 4096 pixels per batch
IW = 68               # padded image width (66 needed + 2 slack)
NT = NPIX // 128      # 32 pixel tiles
NTK = NT * NK
MAGIC = 1.5 * 2.0**23  # fp32 round-to-int trick
CLIP_MAX = 65.0
NQ = 4                 # SWDGE queues
NWARM_A = 70           # HAM warmup matmuls before the idx-transpose MMs
NWARM_B = 150          # bridge warmups between idx MMs and tile 0

KY = np.array([k // 3 - 1 for k in range(9)], np.float32)
KX = np.array([k % 3 - 1 for k in range(9)], np.float32)


def _host_consts():
    p = np.arange(128)
    t = np.arange(NT)
    h = 2 * t[None, :, None] + (p[:, None, None] // 64)
    w = (p[:, None, None] % 64)
    byp = (h + 1 + KY[None, None, :]).astype(np.float32).reshape(128, NTK)
    bxp = (np.broadcast_to(w, (128, NT, 1)) + 1 + KX[None, None, :]).astype(
        np.float32
    ).reshape(128, NTK)
    bcat = np.concatenate([byp, bxp], axis=1)  # [128, 2*NTK]

    # diagonal all-ones-bits mask, bf16 container
    mask = np.where(np.eye(128, dtype=bool), np.uint16(0xFFFF), np.uint16(0))
    import ml_dtypes

    return {
        "bcat": bcat,
        "mask": mask.view(ml_dtypes.bfloat16),
        "identf": np.eye(128, dtype=np.float32),
    }


def build_nc():
    from concourse.bacc import Bacc

    nc = Bacc(num_swdge_queues=NQ)

    x = nc.dram_tensor("x", [NPIX, C], F32, kind="ExternalInput")
    off = nc.dram_tensor("off", [NPIX, 2 * NK], F32, kind="ExternalInput")
    mod = nc.dram_tensor("mod", [NPIX, NK], F32, kind="ExternalInput")
    ck = nc.dram_tensor("ck", [NK, C, F], F32, kind="ExternalInput")
    bcat_d = nc.dram_tensor("bcat", [128, 2 * NTK], F32, kind="ExternalInput")
    mask_d = nc.dram_tensor("mask", [128, 128], BF16, kind="ExternalInput")
    identf_d = nc.dram_tensor("identf", [128, 128], F32, kind="ExternalInput")
    out = nc.dram_tensor("out", [NPIX, F], F32, kind="ExternalOutput")

    # img2[j*IW+xx] = (img[j-1, xx], img[j, xx]) fp8 row pairs; j = ye+1
    img2 = nc.dram_tensor("img2", [67 * IW, 2 * C], FP8, kind="Internal")
    zs = nc.dram_tensor("zs", [35840], FP8, kind="Internal")
    idxd = nc.dram_tensor("idxd", [16 * NT * 72], I16, kind="Internal")

    with TileContext(nc) as tc:
        _body(tc, x, off, mod, ck, bcat_d, mask_d, identf_d, out, img2, zs, idxd)
    nc.finalize()
    return nc


def _body(tc, x, off, mod, ck, bcat_d, mask_d, identf_d, out, img2, zs, idxd):
    import contextlib

    nc = tc.nc
    alu = mybir.AluOpType
    act_copy = mybir.ActivationFunctionType.Copy
    ctx = contextlib.ExitStack()
    cpool = ctx.enter_context(tc.tile_pool(name="consts", bufs=1))
    spool = ctx.enter_context(tc.tile_pool(name="setup", bufs=1))
    gpool = ctx.enter_context(tc.tile_pool(name="gather", bufs=9))
    dgpool = ctx.enter_context(tc.tile_pool(name="diag", bufs=2))
    opool = ctx.enter_context(tc.tile_pool(name="outs", bufs=2))
    ppool = ctx.enter_context(tc.tile_pool(name="psum", bufs=3, space="PSUM"))
    p2pool = ctx.enter_context(tc.tile_pool(name="psum2", bufs=2, space="PSUM"))
    wmpool = ctx.enter_context(tc.tile_pool(name="psumw", bufs=1, space="PSUM"))

    # ---------------- persistent tiles ----------------
    bcat = cpool.tile([128, 2 * NTK], F32, name="bcatt")
    mask = cpool.tile([128, 128], BF16, name="maskt")
    kmat = cpool.tile([128, NK * F], BF16, name="kmatt")
    w4dup = cpool.tile([128, NTK * 4 * 2], BF16, name="w4dup")
    idx = cpool.tile([128, NT * 72], I16, name="idx")
    feats = cpool.tile([128, 3 * 192 * 64], BF16, name="featsbuf")
    zt = cpool.tile([128, 280], FP8, name="zt")

    # ---------------- input loads (offsets first, split over 2 queues)
    offv = off[:].rearrange("(t p) c -> p t c", p=128)
    offp = spool.tile([128, NT, 2 * NK], F32, name="offp")
    nc.sync.dma_start(out=offp[:, 0 : NT // 2, :], in_=offv[:, 0 : NT // 2, :])
    nc.scalar.dma_start(
        out=offp[:, NT // 2 : NT, :], in_=offv[:, NT // 2 : NT, :]
    )

    # ---------------- Stage A: padded fp8 image ----------------------
    # zeros staging: memset SBUF, park in DRAM for border fills
    nc.vector.memset(zt[:], 0.0)
    nc.sync.dma_start(
        out=zs[:].rearrange("(p n) -> p n", n=280), in_=zt[:]
    )
    nc.sync.dma_start(out=mask[:], in_=mask_d[:])
    identf = cpool.tile([128, 128], F32, name="identft")
    nc.sync.dma_start(out=identf[:], in_=identf_d[:])
    nc.sync.dma_start(out=bcat[:], in_=bcat_d[:])

    i2v = img2[:].rearrange("(j xx) (s c) -> j xx s c", xx=IW, s=2)
    # interior: slot0 <- x rows (j-1 in 1..64), slot1 <- x rows (j in 1..64)
    xv = x[:].rearrange("(h w) c -> h w c", w=W)
    nc.gpsimd.dma_start(out=i2v[2:66, 1:65, 0, :], in_=xv)
    nc.gpsimd.dma_start(out=i2v[1:65, 1:65, 1, :], in_=xv)
    # borders (zero): full rows j=0 (both slots), j=65 slot1 + j=66 both,
    # j=1 slot0, plus cols xx in {0,65,66,67} for all j.
    nc.scalar.dma_start(out=img2[0:IW, :], in_=zs[0 : IW * 256])          # j=0
    nc.scalar.dma_start(out=i2v[1, :, 0, :], in_=zs[0 : IW * 128])        # j=1 s0
    nc.scalar.dma_start(out=i2v[65, :, 1, :], in_=zs[0 : IW * 128])       # j=65 s1
    nc.scalar.dma_start(out=img2[66 * IW : 67 * IW, :], in_=zs[0 : IW * 256])
    zcol = bass.AP(zs[:].tensor, 0, [[0, 67], [1, 256]])
    nc.scalar.dma_start(
        out=bass.AP(img2[:].tensor, 0, [[IW * 256, 67], [1, 256]]), in_=zcol
    )  # xx=0, all j, both slots
    zcol3 = bass.AP(zs[:].tensor, 0, [[0, 67], [1, 3 * 256]])
    nc.scalar.dma_start(
        out=bass.AP(img2[:].tensor, 65 * 256, [[IW * 256, 67], [1, 3 * 256]]),
        in_=zcol3,
    )  # xx=65..67

    # dummy 1-row gather from zs: forces the Q7 SWDGE library load (~9us)
    # off the critical path, long before the first real gather.
    zidx = cpool.tile([128, 8], I16, name="zidx")
    nc.vector.memset(zidx[:], 0)
    zdst = cpool.tile([128, 512], FP8, name="zdst")
    nc.gpsimd.dma_gather(
        zdst[:].rearrange("p (a b) -> p a b", a=1),
        bass.AP(zs[:].tensor, 0, [[256, 66], [1, 512]]),
        zidx[:, 0:8],
        num_idxs=128,
        num_idxs_reg=128,
        elem_size=512,
        elem_step=256,
        single_packet=False,
        queue_num=0,
    )

    # ---------------- warmup matmuls phase A (keep HAM at 8/8) --------
    pw = wmpool.tile([128, 128], F32, name="pwarm")
    for _ in range(NWARM_A):
        nc.tensor.matmul(pw[:], mask[:], mask[:], start=True, stop=True)

    # modulation load on the scalar queue (needed only by the w4 chain)
    modv = mod[:].rearrange("(t p) c -> p t c", p=128)
    modp = spool.tile([128, NT, NK], F32, name="modp")
    nc.scalar.dma_start(out=modp[:], in_=modv)
    modf = modp[:].rearrange("p t k -> p (t k)")

    # ---------------- Stage B: pix-major index + weight math ----------
    def st(name, n=NTK):
        return spool.tile([128, n], F32, name=name)

    # pypx[0:NTK]=py, [NTK:2NTK]=px: one add of offp (strided view) + bcat
    pypx = st("pypx", 2 * NTK)
    offs = bass.AP(
        offp[:].tensor,
        offp[:].offset,
        [list(offp[:].ap[0]), [NK, 2], [2 * NK, NT], [1, NK]],
    )
    nc.vector.tensor_tensor(
        pypx[:].rearrange("p (h t k) -> p h t k", h=2, k=NK), offs, bcat[:].rearrange("p (h t k) -> p h t k", h=2, k=NK), alu.add
    )
    py = pypx[:, 0:NTK]
    px = pypx[:, NTK : 2 * NTK]

    # floor via magic-number round trick, both halves at once
    y0fx0f = st("y0fx0f", 2 * NTK)
    nc.vector.tensor_scalar(y0fx0f[:], pypx[:], -0.5, MAGIC, alu.add, alu.add)
    nc.vector.tensor_scalar(y0fx0f[:], y0fx0f[:], MAGIC, None, alu.subtract)
    y0f = y0fx0f[:, 0:NTK]
    x0f = y0fx0f[:, NTK : 2 * NTK]

    # --- idx chain (feeds the gathers; do first) ---
    # y0x0c holds [y0 | x0] clipped at 0; ye = clip(y0f, -1, CLIP)
    y0x0 = st("y0x0", 2 * NTK)
    nc.vector.tensor_scalar(y0x0[:], y0fx0f[:], 0.0, CLIP_MAX, alu.max, alu.min)
    x06 = y0x0[:, NTK : 2 * NTK]
    ye = st("ye")
    nc.vector.tensor_scalar(ye[:], y0f, -1.0, CLIP_MAX, alu.max, alu.min)
    i0f = st("i0f")
    nc.vector.tensor_scalar(i0f[:], ye[:], float(IW), float(IW), alu.mult, alu.add)
    nc.vector.tensor_add(i0f[:], i0f[:], x06)
    # PE-transpose pix-major [p=(g,q), (t,k)] to 16-part gather layout:
    # out_g[q, (t,k)] = i0f[g*16+q, (t,k)] via identity-column selector.
    # DVE then interleaves (t,k,g) while casting f32 -> i16; DRAM write and
    # replicated read-back are both fully contiguous.
    i16q = spool.tile([128, NT * 72], I16, name="i16q")
    # stage the selector-MM outputs to SBUF via ACT (scalar engine) so the
    # DVE casts don't ping-pong with the f32 matmuls through 2 PSUM bufs
    i0s = spool.tile([128, 8, NTK], F32, name="i0s")
    for g in range(8):
        itp = ppool.tile([128, NTK], F32, name="itp", tag="itp", bufs=2)
        nc.tensor.matmul(
            itp[0:16, :], identf[:, g * 16 : (g + 1) * 16], i0f[:],
            start=True, stop=True,
        )
        nc.scalar.activation(i0s[0:16, g, :], itp[0:16, :], act_copy)
    # chunked [16,72] interleave casts run at ~320ns each vs ~3us full-range
    TPCH = 8
    for g in range(8):
        for c in range(4):
            t0 = c * TPCH
            dst = bass.AP(
                i16q.tensor,
                i16q.offset + t0 * 72 + g,
                [[i16q.ap[0][0], 16], [72, TPCH], [8, NK]],
            )
            nc.vector.tensor_copy(
                dst, i0s[0:16, g, t0 * NK : (t0 + TPCH) * NK]
            )
    nc.scalar.dma_start(
        out=idxd[:].rearrange("(q n) -> q n", n=NT * 72), in_=i16q[0:16, :]
    )
    # replicated read back: all 128 partitions = 8 copies of the 16-part set.
    # On the gpsimd SWDGE queue: descriptor generation there is ~100x faster
    # than on the sync/scalar HWDGE queues (observed 0.5ns vs 90ns per desc).
    nc.gpsimd.dma_start(
        out=idx[:],
        in_=bass.AP(idxd[:].tensor, 0, [[0, 8], [NT * 72, 16], [1, NT * 72]]),
    )

    # warmup phase B: bridge the PE from idx MMs to tile 0's matmuls
    for _ in range(NWARM_B):
        nc.tensor.matmul(pw[:], mask[:], mask[:], start=True, stop=True)

    # ---------------- gathers: enqueue all 32 now -------------------
    img_gsrc = bass.AP(img2[:].tensor, 0, [[256, 67 * IW - 1], [1, 512]])
    gtiles = []
    for call in range(NT):
        gg = gpool.tile([128, NK, 512], FP8, name="gt")
        nc.gpsimd.dma_gather(
            gg[:],
            img_gsrc,
            idx[:, call * 72 : (call + 1) * 72],
            num_idxs=NK * 128,
            num_idxs_reg=NK * 128,
            elem_size=512,
            elem_step=256,
            single_packet=False,
            queue_num=(0, 2, 1, 3)[call % NQ],
        )
        gtiles.append(gg)

    # ---------------- weight chain (w4) ------------------------------
    y0 = y0x0[:, 0:NTK]
    x1c = st("x1c")
    nc.vector.tensor_scalar(x1c[:], x0f, 1.0, 0.0, alu.add, alu.max)
    nc.vector.tensor_scalar(x1c[:], x1c[:], CLIP_MAX, None, alu.min)

    pycpxc = st("pycpxc", 2 * NTK)
    nc.vector.tensor_scalar(pycpxc[:], pypx[:], 0.0, CLIP_MAX, alu.max, alu.min)
    lylx = st("lylx", 2 * NTK)
    nc.vector.tensor_sub(lylx[:], pycpxc[:], y0x0[:])
    ly = lylx[:, 0:NTK]
    lx = lylx[:, NTK : 2 * NTK]
    sx = st("sx")
    nc.vector.tensor_sub(sx[:], x1c[:], x06)

    olyolx = st("olyolx", 2 * NTK)
    nc.vector.tensor_scalar(olyolx[:], lylx[:], -1.0, 1.0, alu.mult, alu.add)
    oly = olyolx[:, 0:NTK]
    olx = olyolx[:, NTK : 2 * NTK]

    am = st("am")
    bm = st("bm")
    nc.vector.tensor_mul(am[:], olx, modf)
    nc.vector.tensor_mul(bm[:], lx, modf)
    a0 = st("a0")
    b0 = st("b0")
    a1 = st("a1")
    b1 = st("b1")
    nc.vector.tensor_mul(a0[:], oly, am[:])
    nc.vector.tensor_mul(b0[:], ly, am[:])
    nc.vector.tensor_mul(a1[:], oly, bm[:])
    nc.vector.tensor_mul(b1[:], ly, bm[:])

    osx = st("osx")
    nc.vector.tensor_scalar(osx[:], sx[:], -1.0, 1.0, alu.mult, alu.add)

    # w4 slots (r0x0, r1x0, r0x1, r1x1); x1 slots gated by sx
    w4 = spool.tile([128, NTK, 4], F32, name="w4")
    tmp = st("tmpw")
    nc.vector.tensor_mul(tmp[:], osx[:], b0[:])
    nc.vector.tensor_add(w4[:, :, 0], tmp[:], a0[:])
    nc.vector.tensor_mul(tmp[:], osx[:], b1[:])
    nc.vector.tensor_add(w4[:, :, 1], tmp[:], a1[:])
    nc.vector.tensor_mul(w4[:, :, 2], sx[:], b0[:])
    nc.vector.tensor_mul(w4[:, :, 3], sx[:], b1[:])
    # duplicate each weight twice (uint32 AND packing)
    w4df = w4dup[:].rearrange("p (a b) -> p a b", b=2)
    w4s = w4[:].rearrange("p a c -> p (a c)")
    w4bc = bass.AP(w4s.tensor, w4s.offset, [list(w4s.ap[0]), list(w4s.ap[1]), [0, 2]])
    nc.vector.tensor_copy(w4df, w4bc)

    # conv kernel f32 load (sync ring) + DVE cast to bf16
    ckstage = dgpool.tile([128, NK * 4, 128], BF16, name="dg")
    ckf32 = ckstage[:].rearrange("p a b -> p (a b)").bitcast(F32)  # [128, 2304]
    nc.sync.dma_start(out=ckf32, in_=ck[:].rearrange("k c f -> c k f"))
    nc.vector.tensor_copy(kmat[:], ckf32)

    # ---------------- Stage C: gather + weighted bilinear transpose ---
    w32all = w4dup[:].bitcast(U32)  # [128, NTK*4]
    m32 = mask[:].bitcast(U32)      # [128, 64]
    m32b = bass.AP(m32.tensor, m32.offset,
                   [list(m32.ap[0]), [0, NK * 4], list(m32.ap[1])])

    def q(i):
        return (i // 6) * 6 + (i % 3) * 2 + ((i % 6) // 3)

    def tile_compute(t, g):
        # dg[p, (k,cr), :] = mask row * w4[p, (t,k,cr)] via uint32 AND
        dg = dgpool.tile([128, NK * 4, 128], BF16, name="dg")
        dg32 = dg[:].bitcast(U32)
        w32 = bass.AP(
            w32all.tensor,
            w32all.offset + t * NK * 4,
            [list(w32all.ap[0]), [1, NK * 4], [0, 64]],
        )
        nc.vector.tensor_tensor(dg32, m32b, w32, alu.bitwise_and)

        for s in range(3):
            pf = ppool.tile([128, 3, 128], F32, name="pfeats")
            for j in range(3):
                k = 3 * s + j
                for cr in range(4):
                    nc.tensor.matmul(
                        pf[:, j, :],
                        g[:, k, cr * 128 : (cr + 1) * 128],
                        dg[:, k * 4 + cr, :],
                        start=(cr == 0),
                        stop=(cr == 3),
                    )
            i0_ = s * 64 + 2 * t
            q0, q1 = q(i0_), q(i0_ + 1)
            # dst: feats[:, j*12288 + q0*64 + {0,(q1-q0)*64} + 0..64] for j=0..2
            dstap = bass.AP(
                feats.tensor,
                feats.offset + q0 * 64,
                [list(feats.ap[0]), [12288, 3], [(q1 - q0) * 64, 2], [1, 64]],
            )
            nc.scalar.activation(dstap, pf[:], act_copy)

        # conv for output tile T once its three feats tiles are written
        for T in range(NT):
            if max((3 * T + u) % NT for u in range(3)) != t:
                continue
            po = p2pool.tile([128, F], F32, name="pout")
            n = 0
            for r in range(3):
                for j in range(3):
                    base = j * 12288 + (T * 6 + r * 2) * 64
                    lhsT = feats[:, base : base + 128]
                    nc.tensor.matmul(
                        po[:],
                        lhsT,
                        kmat[:, (r * 3 + j) * F : (r * 3 + j + 1) * F],
                        start=(n == 0),
                        stop=(n == 8),
                    )
                    n += 1
            ot = opool.tile([128, F], F32, name="ot")
            nc.scalar.activation(ot[:], po[:], act_copy)
            nc.sync.dma_start(out=out[T * 128 : (T + 1) * 128, :], in_=ot[:])

    for t in range(NT):
        tile_compute(t, gtiles[t][:])
    ctx.close()


_CACHED_NC = None


def _get_nc():
    global _CACHED_NC
    if _CACHED_NC is None:
        _CACHED_NC = build_nc()
    return _CACHED_NC


def kernel(x, offset, modulation, conv_kernel, conv_bias):
    from concourse.bass_utils import run_bass_kernel_spmd

    B = x.shape[0]
    consts = _host_consts()
    ck9 = np.ascontiguousarray(conv_kernel.reshape(NK, C, F), dtype=np.float32)
    in_maps = []
    for b in range(B):
        in_maps.append(
            {
                "x": np.ascontiguousarray(x[b].reshape(NPIX, C), np.float32),
                "off": np.ascontiguousarray(
                    offset[b].reshape(NPIX, 2 * NK), np.float32
                ),
                "mod": np.ascontiguousarray(
                    modulation[b].reshape(NPIX, NK), np.float32
                ),
                "ck": ck9,
                "bcat": consts["bcat"],
                "mask": consts["mask"],
                "identf": consts["identf"],
            }
        )
    nc = _get_nc()
    res = run_bass_kernel_spmd(
        nc,
        in_maps,
        core_ids=list(range(B)),
        trace=bool(int(os.environ.get("KERNEL_TRACE", "0"))),
    )
    outs = [res.results[b]["out"].reshape(H, W, F) for b in range(B)]
    result = np.stack(outs, axis=0) + conv_bias[None, None, None, :]
    if getattr(res, "exec_time_ns", None):
        kernel.last_exec_time_ns = res.exec_time_ns
    return result.astype(np.float32)


# revision 28
# speedup vs baseline: 1.0107x; 1.0107x over previous
"""DCNv2 Trainium2 Bass kernel, v3.

Problem: x[8,64,64,128], offset[8,64,64,18], modulation[8,64,64,9],
conv_kernel[3,3,128,256], conv_bias[256] -> out[8,64,64,256].
Data-parallel over batch B=8, one batch per NeuronCore.

v3 changes vs v2:
  - idx pipeline computed pix-major inside the B1 chain (reuses y0f/x0f),
    cast to i16 and rearranged to the 16-partition gather layout via two
    small scalar-queue DMAs.  Drops the whole emit_b2_chunk pipeline
    (saves ~20us serial DVE + ~16us strided loads off the critical path).
  - byp/bxp fused into one bcat const so py/px compute in one DVE op.
  - all 32 gathers enqueued immediately after idx is ready (bufs=9).
  - PSUM batching: 3 taps (one kernel-row s) share one PSUM bank; one
    ACT copy of [128,3,2,64] per (t,s) instead of 3 copies of [128,128].
  - ~250 warmup matmuls on the mask const during setup keep the PE HAM
    at 8/8 so tile 0 starts warm.
"""

import os
import sys

import numpy as np

sys.path.insert(0, "/opt/trn_rl_repo")

import concourse.bass as bass  # noqa: E402
import concourse.mybir as mybir  # noqa: E402
from concourse.tile import TileContext  # noqa: E402

F32 = mybir.dt.float32
BF16 = mybir.dt.bfloat16
FP8 = mybir.dt.float8e3
U32 = mybir.dt.uint32
I16 = mybir.dt.int16

H = W = 64
C = 128
F = 256
NK = 9
NPIX = H * W          # 4096 pixels per batch
IW = 68               # padded image width (66 needed + 2 slack)
NT = NPIX // 128      # 32 pixel tiles
NTK = NT * NK
MAGIC = 1.5 * 2.0**23  # fp32 round-to-int trick
CLIP_MAX = 65.0
NQ = 4                 # SWDGE queues
NWARM_A = 70           # HAM warmup matmuls before the idx-transpose MMs
NWARM_B = 150          # bridge warmups between idx MMs and tile 0

KY = np.array([k // 3 - 1 for k in range(9)], np.float32)
KX = np.array([k % 3 - 1 for k in range(9)], np.float32)


def _host_consts():
    p = np.arange(128)
    t = np.arange(NT)
    h = 2 * t[None, :, None] + (p[:, None, None] // 64)
    w = (p[:, None, None] % 64)
    byp = (h + 1 + KY[None, None, :]).astype(np.float32).reshape(128, NTK)
    bxp = (np.broadcast_to(w, (128, NT, 1)) + 1 + KX[None, None, :]).astype(
        np.float32
    ).reshape(128, NTK)
    bcat = np.concatenate([byp, bxp], axis=1)  # [128, 2*NTK]

    # diagonal all-ones-bits mask, bf16 container
    mask = np.where(np.eye(128, dtype=bool), np.uint16(0xFFFF), np.uint16(0))
    import ml_dtypes

    return {
        "bcat": bcat,
        "mask": mask.view(ml_dtypes.bfloat16),
        "identf": np.eye(128, dtype=np.float32),
    }


def build_nc():
    from concourse.bacc import Bacc

    nc = Bacc(num_swdge_queues=NQ)

    x = nc.dram_tensor("x", [NPIX, C], F32, kind="ExternalInput")
    off = nc.dram_tensor("off", [NPIX, 2 * NK], F32, kind="ExternalInput")
    mod = nc.dram_tensor("mod", [NPIX, NK], F32, kind="ExternalInput")
    ck = nc.dram_tensor("ck", [NK, C, F], F32, kind="ExternalInput")
    bcat_d = nc.dram_tensor("bcat", [128, 2 * NTK], F32, kind="ExternalInput")
    mask_d = nc.dram_tensor("mask", [128, 128], BF16, kind="ExternalInput")
    identf_d = nc.dram_tensor("identf", [128, 128], F32, kind="ExternalInput")
    out = nc.dram_tensor("out", [NPIX, F], F32, kind="ExternalOutput")

    # img2[j*IW+xx] = (img[j-1, xx], img[j, xx]) fp8 row pairs; j = ye+1
    img2 = nc.dram_tensor("img2", [67 * IW, 2 * C], FP8, kind="Internal")
    zs = nc.dram_tensor("zs", [35840], FP8, kind="Internal")
    idxd = nc.dram_tensor("idxd", [16 * NT * 72], I16, kind="Internal")

    with TileContext(nc) as tc:
        _body(tc, x, off, mod, ck, bcat_d, mask_d, identf_d, out, img2, zs, idxd)
    nc.finalize()
    return nc


def _body(tc, x, off, mod, ck, bcat_d, mask_d, identf_d, out, img2, zs, idxd):
    import contextlib

    nc = tc.nc
    alu = mybir.AluOpType
    act_copy = mybir.ActivationFunctionType.Copy
    ctx = contextlib.ExitStack()
    cpool = ctx.enter_context(tc.tile_pool(name="consts", bufs=1))
    spool = ctx.enter_context(tc.tile_pool(name="setup", bufs=1))
    gpool = ctx.enter_context(tc.tile_pool(name="gather", bufs=10))
    dgpool = ctx.enter_context(tc.tile_pool(name="diag", bufs=3))
    opool = ctx.enter_context(tc.tile_pool(name="outs", bufs=2))
    ppool = ctx.enter_context(tc.tile_pool(name="psum", bufs=3, space="PSUM"))
    p2pool = ctx.enter_context(tc.tile_pool(name="psum2", bufs=2, space="PSUM"))
    wmpool = ctx.enter_context(tc.tile_pool(name="psumw", bufs=1, space="PSUM"))

    # ---------------- persistent tiles ----------------
    bcat = cpool.tile([128, 2 * NTK], F32, name="bcatt")
    mask = cpool.tile([128, 128], BF16, name="maskt")
    kmat = cpool.tile([128, NK * F], BF16, name="kmatt")
    w4dup = cpool.tile([128, NTK * 4 * 2], BF16, name="w4dup")
    idx = cpool.tile([128, NT * 72], I16, name="idx")
    feats = cpool.tile([128, 3 * 192 * 64], BF16, name="featsbuf")
    zt = cpool.tile([128, 280], FP8, name="zt")

    # ---------------- input loads (offsets first, split over 2 queues)
    offv = off[:].rearrange("(t p) c -> p t c", p=128)
    offp = spool.tile([128, NT, 2 * NK], F32, name="offp")
    nc.sync.dma_start(out=offp[:, 0 : NT // 2, :], in_=offv[:, 0 : NT // 2, :])
    nc.scalar.dma_start(
        out=offp[:, NT // 2 : NT, :], in_=offv[:, NT // 2 : NT, :]
    )

    # ---------------- Stage A: padded fp8 image ----------------------
    # zeros staging: memset SBUF, park in DRAM for border fills
    nc.vector.memset(zt[:], 0.0)
    nc.sync.dma_start(
        out=zs[:].rearrange("(p n) -> p n", n=280), in_=zt[:]
    )
    nc.sync.dma_start(out=mask[:], in_=mask_d[:])
    identf = cpool.tile([128, 128], F32, name="identft")
    nc.sync.dma_start(out=identf[:], in_=identf_d[:])
    nc.sync.dma_start(out=bcat[:], in_=bcat_d[:])

    i2v = img2[:].rearrange("(j xx) (s c) -> j xx s c", xx=IW, s=2)
    # interior: slot0 <- x rows (j-1 in 1..64), slot1 <- x rows (j in 1..64)
    xv = x[:].rearrange("(h w) c -> h w c", w=W)
    nc.gpsimd.dma_start(out=i2v[2:66, 1:65, 0, :], in_=xv)
    nc.gpsimd.dma_start(out=i2v[1:65, 1:65, 1, :], in_=xv)
    # borders (zero): full rows j=0 (both slots), j=65 slot1 + j=66 both,
    # j=1 slot0, plus cols xx in {0,65,66,67} for all j.
    nc.scalar.dma_start(out=img2[0:IW, :], in_=zs[0 : IW * 256])          # j=0
    nc.scalar.dma_start(out=i2v[1, :, 0, :], in_=zs[0 : IW * 128])        # j=1 s0
    nc.scalar.dma_start(out=i2v[65, :, 1, :], in_=zs[0 : IW * 128])       # j=65 s1
    nc.scalar.dma_start(out=img2[66 * IW : 67 * IW, :], in_=zs[0 : IW * 256])
    zcol = bass.AP(zs[:].tensor, 0, [[0, 67], [1, 256]])
    nc.scalar.dma_start(
        out=bass.AP(img2[:].tensor, 0, [[IW * 256, 67], [1, 256]]), in_=zcol
    )  # xx=0, all j, both slots
    zcol3 = bass.AP(zs[:].tensor, 0, [[0, 67], [1, 3 * 256]])
    nc.scalar.dma_start(
        out=bass.AP(img2[:].tensor, 65 * 256, [[IW * 256, 67], [1, 3 * 256]]),
        in_=zcol3,
    )  # xx=65..67

    # dummy 1-row gather from zs: forces the Q7 SWDGE library load (~9us)
    # off the critical path, long before the first real gather.
    zidx = cpool.tile([128, 8], I16, name="zidx")
    nc.vector.memset(zidx[:], 0)
    zdst = cpool.tile([128, 512], FP8, name="zdst")
    nc.gpsimd.dma_gather(
        zdst[:].rearrange("p (a b) -> p a b", a=1),
        bass.AP(zs[:].tensor, 0, [[256, 66], [1, 512]]),
        zidx[:, 0:8],
        num_idxs=128,
        num_idxs_reg=128,
        elem_size=512,
        elem_step=256,
        single_packet=False,
        queue_num=0,
    )

    # ---------------- warmup matmuls phase A (keep HAM at 8/8) --------
    pw = wmpool.tile([128, 128], F32, name="pwarm")
    for _ in range(NWARM_A):
        nc.tensor.matmul(pw[:], mask[:], mask[:], start=True, stop=True)

    # modulation load on the scalar queue (needed only by the w4 chain)
    modv = mod[:].rearrange("(t p) c -> p t c", p=128)
    modp = spool.tile([128, NT, NK], F32, name="modp")
    nc.scalar.dma_start(out=modp[:], in_=modv)
    modf = modp[:].rearrange("p t k -> p (t k)")

    # ---------------- Stage B: pix-major index + weight math ----------
    def st(name, n=NTK):
        return spool.tile([128, n], F32, name=name)

    # pypx[0:NTK]=py, [NTK:2NTK]=px: one add of offp (strided view) + bcat
    pypx = st("pypx", 2 * NTK)
    offs = bass.AP(
        offp[:].tensor,
        offp[:].offset,
        [list(offp[:].ap[0]), [NK, 2], [2 * NK, NT], [1, NK]],
    )
    nc.vector.tensor_tensor(
        pypx[:].rearrange("p (h t k) -> p h t k", h=2, k=NK), offs, bcat[:].rearrange("p (h t k) -> p h t k", h=2, k=NK), alu.add
    )
    py = pypx[:, 0:NTK]
    px = pypx[:, NTK : 2 * NTK]

    # floor via magic-number round trick, both halves at once
    y0fx0f = st("y0fx0f", 2 * NTK)
    nc.vector.tensor_scalar(y0fx0f[:], pypx[:], -0.5, MAGIC, alu.add, alu.add)
    nc.vector.tensor_scalar(y0fx0f[:], y0fx0f[:], MAGIC, None, alu.subtract)
    y0f = y0fx0f[:, 0:NTK]
    x0f = y0fx0f[:, NTK : 2 * NTK]

    # --- idx chain (feeds the gathers; do first) ---
    # y0x0c holds [y0 | x0] clipped at 0; ye = clip(y0f, -1, CLIP)
    y0x0 = st("y0x0", 2 * NTK)
    nc.vector.tensor_scalar(y0x0[:], y0fx0f[:], 0.0, CLIP_MAX, alu.max, alu.min)
    x06 = y0x0[:, NTK : 2 * NTK]
    ye = st("ye")
    nc.vector.tensor_scalar(ye[:], y0f, -1.0, CLIP_MAX, alu.max, alu.min)
    i0f = st("i0f")
    nc.vector.tensor_scalar(i0f[:], ye[:], float(IW), float(IW), alu.mult, alu.add)
    nc.vector.tensor_add(i0f[:], i0f[:], x06)
    # PE-transpose pix-major [p=(g,q), (t,k)] to 16-part gather layout:
    # out_g[q, (t,k)] = i0f[g*16+q, (t,k)] via identity-column selector.
    # DVE then interleaves (t,k,g) while casting f32 -> i16; DRAM write and
    # replicated read-back are both fully contiguous.
    i16q = spool.tile([128, NT * 72], I16, name="i16q")
    for g in range(8):
        itp = ppool.tile([128, NTK], F32, name="itp", tag="itp", bufs=2)
        nc.tensor.matmul(
            itp[0:16, :], identf[:, g * 16 : (g + 1) * 16], i0f[:],
            start=True, stop=True,
        )
        dst = bass.AP(
            i16q.tensor, i16q.offset + g, [[i16q.ap[0][0], 16], [72, NT], [8, NK]]
        )
        nc.vector.tensor_copy(dst, itp[0:16, :])
    nc.scalar.dma_start(
        out=idxd[:].rearrange("(q n) -> q n", n=NT * 72), in_=i16q[0:16, :]
    )
    # replicated read back: all 128 partitions = 8 copies of the 16-part set.
    # On the gpsimd SWDGE queue: descriptor generation there is ~100x faster
    # than on the sync/scalar HWDGE queues (observed 0.5ns vs 90ns per desc).
    nc.gpsimd.dma_start(
        out=idx[:],
        in_=bass.AP(idxd[:].tensor, 0, [[0, 8], [NT * 72, 16], [1, NT * 72]]),
    )

    # warmup phase B: bridge the PE from idx MMs to tile 0's matmuls
    for _ in range(NWARM_B):
        nc.tensor.matmul(pw[:], mask[:], mask[:], start=True, stop=True)

    # ---------------- gathers: enqueue all 32 now -------------------
    img_gsrc = bass.AP(img2[:].tensor, 0, [[256, 67 * IW - 1], [1, 512]])
    gtiles = []
    for call in range(NT):
        gg = gpool.tile([128, NK, 512], FP8, name="gt")
        nc.gpsimd.dma_gather(
            gg[:],
            img_gsrc,
            idx[:, call * 72 : (call + 1) * 72],
            num_idxs=NK * 128,
            num_idxs_reg=NK * 128,
            elem_size=512,
            elem_step=256,
            single_packet=False,
            queue_num=(0, 2, 1, 3)[call % NQ],
        )
        gtiles.append(gg)

    # ---------------- weight chain (w4) ------------------------------
    y0 = y0x0[:, 0:NTK]
    x1c = st("x1c")
    nc.vector.tensor_scalar(x1c[:], x0f, 1.0, 0.0, alu.add, alu.max)
    nc.vector.tensor_scalar(x1c[:], x1c[:], CLIP_MAX, None, alu.min)

    pycpxc = st("pycpxc", 2 * NTK)
    nc.vector.tensor_scalar(pycpxc[:], pypx[:], 0.0, CLIP_MAX, alu.max, alu.min)
    lylx = st("lylx", 2 * NTK)
    nc.vector.tensor_sub(lylx[:], pycpxc[:], y0x0[:])
    ly = lylx[:, 0:NTK]
    lx = lylx[:, NTK : 2 * NTK]
    sx = st("sx")
    nc.vector.tensor_sub(sx[:], x1c[:], x06)

    olyolx = st("olyolx", 2 * NTK)
    nc.vector.tensor_scalar(olyolx[:], lylx[:], -1.0, 1.0, alu.mult, alu.add)
    oly = olyolx[:, 0:NTK]
    olx = olyolx[:, NTK : 2 * NTK]

    am = st("am")
    bm = st("bm")
    nc.vector.tensor_mul(am[:], olx, modf)
    nc.vector.tensor_mul(bm[:], lx, modf)
    a0 = st("a0")
    b0 = st("b0")
    a1 = st("a1")
    b1 = st("b1")
    nc.vector.tensor_mul(a0[:], oly, am[:])
    nc.vector.tensor_mul(b0[:], ly, am[:])
    nc.vector.tensor_mul(a1[:], oly, bm[:])
    nc.vector.tensor_mul(b1[:], ly, bm[:])

    osx = st("osx")
    nc.vector.tensor_scalar(osx[:], sx[:], -1.0, 1.0, alu.mult, alu.add)

    # w4 slots (r0x0, r1x0, r0x1, r1x1); x1 slots gated by sx
    w4 = spool.tile([128, NTK, 4], F32, name="w4")
    tmp = st("tmpw")
    nc.vector.tensor_mul(tmp[:], osx[:], b0[:])
    nc.vector.tensor_add(w4[:, :, 0], tmp[:], a0[:])
    nc.vector.tensor_mul(tmp[:], osx[:], b1[:])
    nc.vector.tensor_add(w4[:, :, 1], tmp[:], a1[:])
    nc.vector.tensor_mul(w4[:, :, 2], sx[:], b0[:])
    nc.vector.tensor_mul(w4[:, :, 3], sx[:], b1[:])
    # duplicate each weight twice (uint32 AND packing)
    w4df = w4dup[:].rearrange("p (a b) -> p a b", b=2)
    w4s = w4[:].rearrange("p a c -> p (a c)")
    w4bc = bass.AP(w4s.tensor, w4s.offset, [list(w4s.ap[0]), list(w4s.ap[1]), [0, 2]])
    nc.vector.tensor_copy(w4df, w4bc)

    # conv kernel f32 load (sync ring) + DVE cast to bf16
    ckstage = dgpool.tile([128, NK * 4, 128], BF16, name="dg")
    ckf32 = ckstage[:].rearrange("p a b -> p (a b)").bitcast(F32)  # [128, 2304]
    nc.sync.dma_start(out=ckf32, in_=ck[:].rearrange("k c f -> c k f"))
    nc.vector.tensor_copy(kmat[:], ckf32)

    # ---------------- Stage C: gather + weighted bilinear transpose ---
    w32all = w4dup[:].bitcast(U32)  # [128, NTK*4]
    m32 = mask[:].bitcast(U32)      # [128, 64]
    m32b = bass.AP(m32.tensor, m32.offset,
                   [list(m32.ap[0]), [0, NK * 4], list(m32.ap[1])])

    def q(i):
        return (i // 6) * 6 + (i % 3) * 2 + ((i % 6) // 3)

    def tile_compute(t, g):
        # dg[p, (k,cr), :] = mask row * w4[p, (t,k,cr)] via uint32 AND
        dg = dgpool.tile([128, NK * 4, 128], BF16, name="dg")
        dg32 = dg[:].bitcast(U32)
        w32 = bass.AP(
            w32all.tensor,
            w32all.offset + t * NK * 4,
            [list(w32all.ap[0]), [1, NK * 4], [0, 64]],
        )
        nc.vector.tensor_tensor(dg32, m32b, w32, alu.bitwise_and)

        for s in range(3):
            pf = ppool.tile([128, 3, 128], F32, name="pfeats")
            for j in range(3):
                k = 3 * s + j
                for cr in range(4):
                    nc.tensor.matmul(
                        pf[:, j, :],
                        g[:, k, cr * 128 : (cr + 1) * 128],
                        dg[:, k * 4 + cr, :],
                        start=(cr == 0),
                        stop=(cr == 3),
                    )
            i0_ = s * 64 + 2 * t
            q0, q1 = q(i0_), q(i0_ + 1)
            # dst: feats[:, j*12288 + q0*64 + {0,(q1-q0)*64} + 0..64] for j=0..2
            dstap = bass.AP(
                feats.tensor,
                feats.offset + q0 * 64,
                [list(feats.ap[0]), [12288, 3], [(q1 - q0) * 64, 2], [1, 64]],
            )
            nc.scalar.activation(dstap, pf[:], act_copy)

        # conv for output tile T once its three feats tiles are written
        for T in range(NT):
            if max((3 * T + u) % NT for u in range(3)) != t:
                continue
            po = p2pool.tile([128, F], F32, name="pout")
            n = 0
            for r in range(3):
                for j in range(3):
                    base = j * 12288 + (T * 6 + r * 2) * 64
                    lhsT = feats[:, base : base + 128]
                    nc.tensor.matmul(
                        po[:],
                        lhsT,
                        kmat[:, (r * 3 + j) * F : (r * 3 + j + 1) * F],
                        start=(n == 0),
                        stop=(n == 8),
                    )
                    n += 1
            ot = opool.tile([128, F], F32, name="ot")
            nc.scalar.activation(ot[:], po[:], act_copy)
            nc.sync.dma_start(out=out[T * 128 : (T + 1) * 128, :], in_=ot[:])

    for t in range(NT):
        tile_compute(t, gtiles[t][:])
    ctx.close()


_CACHED_NC = None


def _get_nc():
    global _CACHED_NC
    if _CACHED_NC is None:
        _CACHED_NC = build_nc()
    return _CACHED_NC


def kernel(x, offset, modulation, conv_kernel, conv_bias):
    from concourse.bass_utils import run_bass_kernel_spmd

    B = x.shape[0]
    consts = _host_consts()
    ck9 = np.ascontiguousarray(conv_kernel.reshape(NK, C, F), dtype=np.float32)
    in_maps = []
    for b in range(B):
        in_maps.append(
            {
                "x": np.ascontiguousarray(x[b].reshape(NPIX, C), np.float32),
                "off": np.ascontiguousarray(
                    offset[b].reshape(NPIX, 2 * NK), np.float32
                ),
                "mod": np.ascontiguousarray(
                    modulation[b].reshape(NPIX, NK), np.float32
                ),
                "ck": ck9,
                "bcat": consts["bcat"],
                "mask": consts["mask"],
                "identf": consts["identf"],
            }
        )
    nc = _get_nc()
    res = run_bass_kernel_spmd(
        nc,
        in_maps,
        core_ids=list(range(B)),
        trace=bool(int(os.environ.get("KERNEL_TRACE", "0"))),
    )
    outs = [res.results[b]["out"].reshape(H, W, F) for b in range(B)]
    result = np.stack(outs, axis=0) + conv_bias[None, None, None, :]
    if getattr(res, "exec_time_ns", None):
        kernel.last_exec_time_ns = res.exec_time_ns
    return result.astype(np.float32)


# revision 30
# speedup vs baseline: 1.0338x; 1.0229x over previous
"""DCNv2 Trainium2 Bass kernel, v3.

Problem: x[8,64,64,128], offset[8,64,64,18], modulation[8,64,64,9],
conv_kernel[3,3,128,256], conv_bias[256] -> out[8,64,64,256].
Data-parallel over batch B=8, one batch per NeuronCore.

v3 changes vs v2:
  - idx pipeline computed pix-major inside the B1 chain (reuses y0f/x0f),
    cast to i16 and rearranged to the 16-partition gather layout via two
    small scalar-queue DMAs.  Drops the whole emit_b2_chunk pipeline
    (saves ~20us serial DVE + ~16us strided loads off the critical path).
  - byp/bxp fused into one bcat const so py/px compute in one DVE op.
  - all 32 gathers enqueued immediately after idx is ready (bufs=9).
  - PSUM batching: 3 taps (one kernel-row s) share one PSUM bank; one
    ACT copy of [128,3,2,64] per (t,s) instead of 3 copies of [128,128].
  - ~250 warmup matmuls on the mask const during setup keep the PE HAM
    at 8/8 so tile 0 starts warm.
"""

import os
import sys

import numpy as np

sys.path.insert(0, "/opt/trn_rl_repo")

import concourse.bass as bass  # noqa: E402
import concourse.mybir as mybir  # noqa: E402
from concourse.tile import TileContext  # noqa: E402

F32 = mybir.dt.float32
BF16 = mybir.dt.bfloat16
FP8 = mybir.dt.float8e3
U32 = mybir.dt.uint32
I16 = mybir.dt.int16

H = W = 64
C = 128
F = 256
NK = 9
NPIX = H * W          # 4096 pixels per batch
IW = 68               # padded image width (66 needed + 2 slack)
NT = NPIX // 128      # 32 pixel tiles
NTK = NT * NK
MAGIC = 1.5 * 2.0**23  # fp32 round-to-int trick
CLIP_MAX = 65.0
NQ = 4                 # SWDGE queues
NWARM_A = 70           # HAM warmup matmuls before the idx-transpose MMs
NWARM_B = 150          # bridge warmups between idx MMs and tile 0

KY = np.array([k // 3 - 1 for k in range(9)], np.float32)
KX = np.array([k % 3 - 1 for k in range(9)], np.float32)


def _host_consts():
    p = np.arange(128)
    t = np.arange(NT)
    h = 2 * t[None, :, None] + (p[:, None, None] // 64)
    w = (p[:, None, None] % 64)
    byp = (h + 1 + KY[None, None, :]).astype(np.float32).reshape(128, NTK)
    bxp = (np.broadcast_to(w, (128, NT, 1)) + 1 + KX[None, None, :]).astype(
        np.float32
    ).reshape(128, NTK)
    bcat = np.concatenate([byp, bxp], axis=1)  # [128, 2*NTK]

    # diagonal all-ones-bits mask, bf16 container
    mask = np.where(np.eye(128, dtype=bool), np.uint16(0xFFFF), np.uint16(0))
    import ml_dtypes

    return {
        "bcat": bcat,
        "mask": mask.view(ml_dtypes.bfloat16),
        "identf": np.eye(128, dtype=np.float32),
    }


def build_nc():
    from concourse.bacc import Bacc

    nc = Bacc(num_swdge_queues=NQ)

    x = nc.dram_tensor("x", [NPIX, C], F32, kind="ExternalInput")
    off = nc.dram_tensor("off", [NPIX, 2 * NK], F32, kind="ExternalInput")
    mod = nc.dram_tensor("mod", [NPIX, NK], F32, kind="ExternalInput")
    ck = nc.dram_tensor("ck", [NK, C, F], F32, kind="ExternalInput")
    bcat_d = nc.dram_tensor("bcat", [128, 2 * NTK], F32, kind="ExternalInput")
    mask_d = nc.dram_tensor("mask", [128, 128], BF16, kind="ExternalInput")
    identf_d = nc.dram_tensor("identf", [128, 128], F32, kind="ExternalInput")
    out = nc.dram_tensor("out", [NPIX, F], F32, kind="ExternalOutput")

    # img2[j*IW+xx] = (img[j-1, xx], img[j, xx]) fp8 row pairs; j = ye+1
    img2 = nc.dram_tensor("img2", [67 * IW, 2 * C], FP8, kind="Internal")
    zs = nc.dram_tensor("zs", [35840], FP8, kind="Internal")
    idxd = nc.dram_tensor("idxd", [16 * NT * 72], I16, kind="Internal")

    with TileContext(nc) as tc:
        _body(tc, x, off, mod, ck, bcat_d, mask_d, identf_d, out, img2, zs, idxd)
    nc.finalize()
    return nc


def _body(tc, x, off, mod, ck, bcat_d, mask_d, identf_d, out, img2, zs, idxd):
    import contextlib

    nc = tc.nc
    alu = mybir.AluOpType
    act_copy = mybir.ActivationFunctionType.Copy
    ctx = contextlib.ExitStack()
    cpool = ctx.enter_context(tc.tile_pool(name="consts", bufs=1))
    spool = ctx.enter_context(tc.tile_pool(name="setup", bufs=1))
    gpool = ctx.enter_context(tc.tile_pool(name="gather", bufs=9))
    dgpool = ctx.enter_context(tc.tile_pool(name="diag", bufs=2))
    opool = ctx.enter_context(tc.tile_pool(name="outs", bufs=2))
    ppool = ctx.enter_context(tc.tile_pool(name="psum", bufs=3, space="PSUM"))
    p2pool = ctx.enter_context(tc.tile_pool(name="psum2", bufs=2, space="PSUM"))
    wmpool = ctx.enter_context(tc.tile_pool(name="psumw", bufs=1, space="PSUM"))

    # ---------------- persistent tiles ----------------
    bcat = cpool.tile([128, 2 * NTK], F32, name="bcatt")
    mask = cpool.tile([128, 128], BF16, name="maskt")
    kmat = cpool.tile([128, NK * F], BF16, name="kmatt")
    w4dup = cpool.tile([128, NTK * 4 * 2], BF16, name="w4dup")
    idx = cpool.tile([128, NT * 72], I16, name="idx")
    feats = cpool.tile([128, 3 * 192 * 64], BF16, name="featsbuf")
    zt = cpool.tile([128, 280], FP8, name="zt")

    # ---------------- input loads (offsets first, split over 2 queues)
    offv = off[:].rearrange("(t p) c -> p t c", p=128)
    offp = spool.tile([128, NT, 2 * NK], F32, name="offp")
    nc.sync.dma_start(out=offp[:, 0 : NT // 2, :], in_=offv[:, 0 : NT // 2, :])
    nc.scalar.dma_start(
        out=offp[:, NT // 2 : NT, :], in_=offv[:, NT // 2 : NT, :]
    )

    # ---------------- Stage A: padded fp8 image ----------------------
    # zeros staging: memset SBUF, park in DRAM for border fills
    nc.vector.memset(zt[:], 0.0)
    nc.sync.dma_start(
        out=zs[:].rearrange("(p n) -> p n", n=280), in_=zt[:]
    )
    nc.sync.dma_start(out=mask[:], in_=mask_d[:])
    identf = cpool.tile([128, 128], F32, name="identft")
    nc.sync.dma_start(out=identf[:], in_=identf_d[:])
    nc.sync.dma_start(out=bcat[:], in_=bcat_d[:])

    i2v = img2[:].rearrange("(j xx) (s c) -> j xx s c", xx=IW, s=2)
    # interior: slot0 <- x rows (j-1 in 1..64), slot1 <- x rows (j in 1..64)
    xv = x[:].rearrange("(h w) c -> h w c", w=W)
    nc.gpsimd.dma_start(out=i2v[2:66, 1:65, 0, :], in_=xv)
    nc.gpsimd.dma_start(out=i2v[1:65, 1:65, 1, :], in_=xv)
    # borders (zero): full rows j=0 (both slots), j=65 slot1 + j=66 both,
    # j=1 slot0, plus cols xx in {0,65,66,67} for all j.
    nc.scalar.dma_start(out=img2[0:IW, :], in_=zs[0 : IW * 256])          # j=0
    nc.scalar.dma_start(out=i2v[1, :, 0, :], in_=zs[0 : IW * 128])        # j=1 s0
    nc.scalar.dma_start(out=i2v[65, :, 1, :], in_=zs[0 : IW * 128])       # j=65 s1
    nc.scalar.dma_start(out=img2[66 * IW : 67 * IW, :], in_=zs[0 : IW * 256])
    zcol = bass.AP(zs[:].tensor, 0, [[0, 67], [1, 256]])
    nc.scalar.dma_start(
        out=bass.AP(img2[:].tensor, 0, [[IW * 256, 67], [1, 256]]), in_=zcol
    )  # xx=0, all j, both slots
    zcol3 = bass.AP(zs[:].tensor, 0, [[0, 67], [1, 3 * 256]])
    nc.scalar.dma_start(
        out=bass.AP(img2[:].tensor, 65 * 256, [[IW * 256, 67], [1, 3 * 256]]),
        in_=zcol3,
    )  # xx=65..67

    # dummy 1-row gather from zs: forces the Q7 SWDGE library load (~9us)
    # off the critical path, long before the first real gather.
    zidx = cpool.tile([128, 8], I16, name="zidx")
    nc.vector.memset(zidx[:], 0)
    zdst = cpool.tile([128, 512], FP8, name="zdst")
    nc.gpsimd.dma_gather(
        zdst[:].rearrange("p (a b) -> p a b", a=1),
        bass.AP(zs[:].tensor, 0, [[256, 66], [1, 512]]),
        zidx[:, 0:8],
        num_idxs=128,
        num_idxs_reg=128,
        elem_size=512,
        elem_step=256,
        single_packet=False,
        queue_num=0,
    )

    # ---------------- warmup matmuls phase A (keep HAM at 8/8) --------
    pw = wmpool.tile([128, 128], F32, name="pwarm")
    for _ in range(NWARM_A):
        nc.tensor.matmul(pw[:], mask[:], mask[:], start=True, stop=True)

    # modulation load on the scalar queue (needed only by the w4 chain)
    modv = mod[:].rearrange("(t p) c -> p t c", p=128)
    modp = spool.tile([128, NT, NK], F32, name="modp")
    nc.scalar.dma_start(out=modp[:], in_=modv)
    modf = modp[:].rearrange("p t k -> p (t k)")

    # ---------------- Stage B: pix-major index + weight math ----------
    def st(name, n=NTK):
        return spool.tile([128, n], F32, name=name)

    # pypx[0:NTK]=py, [NTK:2NTK]=px: one add of offp (strided view) + bcat
    pypx = st("pypx", 2 * NTK)
    offs = bass.AP(
        offp[:].tensor,
        offp[:].offset,
        [list(offp[:].ap[0]), [NK, 2], [2 * NK, NT], [1, NK]],
    )
    nc.vector.tensor_tensor(
        pypx[:].rearrange("p (h t k) -> p h t k", h=2, k=NK), offs, bcat[:].rearrange("p (h t k) -> p h t k", h=2, k=NK), alu.add
    )
    py = pypx[:, 0:NTK]
    px = pypx[:, NTK : 2 * NTK]

    # floor via magic-number round trick, both halves at once
    y0fx0f = st("y0fx0f", 2 * NTK)
    nc.vector.tensor_scalar(y0fx0f[:], pypx[:], -0.5, MAGIC, alu.add, alu.add)
    nc.vector.tensor_scalar(y0fx0f[:], y0fx0f[:], MAGIC, None, alu.subtract)
    y0f = y0fx0f[:, 0:NTK]
    x0f = y0fx0f[:, NTK : 2 * NTK]

    # --- idx chain (feeds the gathers; do first) ---
    # y0x0c holds [y0 | x0] clipped at 0; ye = clip(y0f, -1, CLIP)
    y0x0 = st("y0x0", 2 * NTK)
    nc.vector.tensor_scalar(y0x0[:], y0fx0f[:], 0.0, CLIP_MAX, alu.max, alu.min)
    x06 = y0x0[:, NTK : 2 * NTK]
    ye = st("ye")
    nc.vector.tensor_scalar(ye[:], y0f, -1.0, CLIP_MAX, alu.max, alu.min)
    i0f = st("i0f")
    nc.vector.tensor_scalar(i0f[:], ye[:], float(IW), float(IW), alu.mult, alu.add)
    nc.vector.tensor_add(i0f[:], i0f[:], x06)
    # PE-transpose pix-major [p=(g,q), (t,k)] to 16-part gather layout:
    # out_g[q, (t,k)] = i0f[g*16+q, (t,k)] via identity-column selector.
    # DVE then interleaves (t,k,g) while casting f32 -> i16; DRAM write and
    # replicated read-back are both fully contiguous.
    i16q = spool.tile([128, NT * 72], I16, name="i16q")
    for g in range(8):
        itp = ppool.tile([128, NTK], F32, name="itp", tag="itp", bufs=2)
        nc.tensor.matmul(
            itp[0:16, :], identf[:, g * 16 : (g + 1) * 16], i0f[:],
            start=True, stop=True,
        )
        dst = bass.AP(
            i16q.tensor, i16q.offset + g, [[i16q.ap[0][0], 16], [72, NT], [8, NK]]
        )
        nc.vector.tensor_copy(dst, itp[0:16, :])
    nc.scalar.dma_start(
        out=idxd[:].rearrange("(q n) -> q n", n=NT * 72), in_=i16q[0:16, :]
    )
    # replicated read back: all 128 partitions = 8 copies of the 16-part set.
    # On the gpsimd SWDGE queue: descriptor generation there is ~100x faster
    # than on the sync/scalar HWDGE queues (observed 0.5ns vs 90ns per desc).
    nc.gpsimd.dma_start(
        out=idx[:],
        in_=bass.AP(idxd[:].tensor, 0, [[0, 8], [NT * 72, 16], [1, NT * 72]]),
    )

    # warmup phase B: bridge the PE from idx MMs to tile 0's matmuls
    for _ in range(NWARM_B):
        nc.tensor.matmul(pw[:], mask[:], mask[:], start=True, stop=True)

    # ---------------- gathers: enqueue all 32 now -------------------
    img_gsrc = bass.AP(img2[:].tensor, 0, [[256, 67 * IW - 1], [1, 512]])
    gtiles = []
    for call in range(NT):
        gg = gpool.tile([128, NK, 512], FP8, name="gt")
        if call == 0:
            # split tile 0's gather so its first PSUM group (taps 0-2) can
            # start ~5us before the full 9-tap descriptor gen completes
            for k0, k1 in ((0, 5), (5, NK)):
                nc.gpsimd.dma_gather(
                    gg[:, k0:k1, :],
                    img_gsrc,
                    idx[:, k0 * 8 : k1 * 8],
                    num_idxs=(k1 - k0) * 128,
                    num_idxs_reg=(k1 - k0) * 128,
                    elem_size=512,
                    elem_step=256,
                    single_packet=False,
                    queue_num=0,
                )
        else:
            nc.gpsimd.dma_gather(
                gg[:],
                img_gsrc,
                idx[:, call * 72 : (call + 1) * 72],
                num_idxs=NK * 128,
                num_idxs_reg=NK * 128,
                elem_size=512,
                elem_step=256,
                single_packet=False,
                queue_num=(0, 2, 1, 3)[call % NQ],
            )
        gtiles.append(gg)

    # ---------------- weight chain (w4) ------------------------------
    y0 = y0x0[:, 0:NTK]
    x1c = st("x1c")
    nc.vector.tensor_scalar(x1c[:], x0f, 1.0, 0.0, alu.add, alu.max)
    nc.vector.tensor_scalar(x1c[:], x1c[:], CLIP_MAX, None, alu.min)

    pycpxc = st("pycpxc", 2 * NTK)
    nc.vector.tensor_scalar(pycpxc[:], pypx[:], 0.0, CLIP_MAX, alu.max, alu.min)
    lylx = st("lylx", 2 * NTK)
    nc.vector.tensor_sub(lylx[:], pycpxc[:], y0x0[:])
    ly = lylx[:, 0:NTK]
    lx = lylx[:, NTK : 2 * NTK]
    sx = st("sx")
    nc.vector.tensor_sub(sx[:], x1c[:], x06)

    olyolx = st("olyolx", 2 * NTK)
    nc.vector.tensor_scalar(olyolx[:], lylx[:], -1.0, 1.0, alu.mult, alu.add)
    oly = olyolx[:, 0:NTK]
    olx = olyolx[:, NTK : 2 * NTK]

    am = st("am")
    bm = st("bm")
    nc.vector.tensor_mul(am[:], olx, modf)
    nc.vector.tensor_mul(bm[:], lx, modf)
    a0 = st("a0")
    b0 = st("b0")
    a1 = st("a1")
    b1 = st("b1")
    nc.vector.tensor_mul(a0[:], oly, am[:])
    nc.vector.tensor_mul(b0[:], ly, am[:])
    nc.vector.tensor_mul(a1[:], oly, bm[:])
    nc.vector.tensor_mul(b1[:], ly, bm[:])

    osx = st("osx")
    nc.vector.tensor_scalar(osx[:], sx[:], -1.0, 1.0, alu.mult, alu.add)

    # w4 slots (r0x0, r1x0, r0x1, r1x1); x1 slots gated by sx
    w4 = spool.tile([128, NTK, 4], F32, name="w4")
    tmp = st("tmpw")
    nc.vector.tensor_mul(tmp[:], osx[:], b0[:])
    nc.vector.tensor_add(w4[:, :, 0], tmp[:], a0[:])
    nc.vector.tensor_mul(tmp[:], osx[:], b1[:])
    nc.vector.tensor_add(w4[:, :, 1], tmp[:], a1[:])
    nc.vector.tensor_mul(w4[:, :, 2], sx[:], b0[:])
    nc.vector.tensor_mul(w4[:, :, 3], sx[:], b1[:])
    # duplicate each weight twice (uint32 AND packing)
    w4df = w4dup[:].rearrange("p (a b) -> p a b", b=2)
    w4s = w4[:].rearrange("p a c -> p (a c)")
    w4bc = bass.AP(w4s.tensor, w4s.offset, [list(w4s.ap[0]), list(w4s.ap[1]), [0, 2]])
    nc.vector.tensor_copy(w4df, w4bc)

    # conv kernel f32 load (sync ring) + DVE cast to bf16
    ckstage = dgpool.tile([128, NK * 4, 128], BF16, name="dg")
    ckf32 = ckstage[:].rearrange("p a b -> p (a b)").bitcast(F32)  # [128, 2304]
    nc.sync.dma_start(out=ckf32, in_=ck[:].rearrange("k c f -> c k f"))
    nc.vector.tensor_copy(kmat[:], ckf32)

    # ---------------- Stage C: gather + weighted bilinear transpose ---
    w32all = w4dup[:].bitcast(U32)  # [128, NTK*4]
    m32 = mask[:].bitcast(U32)      # [128, 64]
    m32b = bass.AP(m32.tensor, m32.offset,
                   [list(m32.ap[0]), [0, NK * 4], list(m32.ap[1])])

    def q(i):
        return (i // 6) * 6 + (i % 3) * 2 + ((i % 6) // 3)

    def tile_compute(t, g):
        # dg[p, (k,cr), :] = mask row * w4[p, (t,k,cr)] via uint32 AND
        dg = dgpool.tile([128, NK * 4, 128], BF16, name="dg")
        dg32 = dg[:].bitcast(U32)
        w32 = bass.AP(
            w32all.tensor,
            w32all.offset + t * NK * 4,
            [list(w32all.ap[0]), [1, NK * 4], [0, 64]],
        )
        nc.vector.tensor_tensor(dg32, m32b, w32, alu.bitwise_and)

        for s in range(3):
            pf = ppool.tile([128, 3, 128], F32, name="pfeats")
            for j in range(3):
                k = 3 * s + j
                for cr in range(4):
                    nc.tensor.matmul(
                        pf[:, j, :],
                        g[:, k, cr * 128 : (cr + 1) * 128],
                        dg[:, k * 4 + cr, :],
                        start=(cr == 0),
                        stop=(cr == 3),
                    )
            i0_ = s * 64 + 2 * t
            q0, q1 = q(i0_), q(i0_ + 1)
            # dst: feats[:, j*12288 + q0*64 + {0,(q1-q0)*64} + 0..64] for j=0..2
            dstap = bass.AP(
                feats.tensor,
                feats.offset + q0 * 64,
                [list(feats.ap[0]), [12288, 3], [(q1 - q0) * 64, 2], [1, 64]],
            )
            nc.scalar.activation(dstap, pf[:], act_copy)

        # conv for output tile T once its three feats tiles are written
        for T in range(NT):
            if max((3 * T + u) % NT for u in range(3)) != t:
                continue
            po = p2pool.tile([128, F], F32, name="pout")
            n = 0
            for r in range(3):
                for j in range(3):
                    base = j * 12288 + (T * 6 + r * 2) * 64
                    lhsT = feats[:, base : base + 128]
                    nc.tensor.matmul(
                        po[:],
                        lhsT,
                        kmat[:, (r * 3 + j) * F : (r * 3 + j + 1) * F],
                        start=(n == 0),
                        stop=(n == 8),
                    )
                    n += 1
            ot = opool.tile([128, F], F32, name="ot")
            nc.scalar.activation(ot[:], po[:], act_copy)
            nc.sync.dma_start(out=out[T * 128 : (T + 1) * 128, :], in_=ot[:])

    for t in range(NT):
        tile_compute(t, gtiles[t][:])
    ctx.close()


_CACHED_NC = None


def _get_nc():
    global _CACHED_NC
    if _CACHED_NC is None:
        _CACHED_NC = build_nc()
    return _CACHED_NC


def kernel(x, offset, modulation, conv_kernel, conv_bias):
    from concourse.bass_utils import run_bass_kernel_spmd

    B = x.shape[0]
    consts = _host_consts()
    ck9 = np.ascontiguousarray(conv_kernel.reshape(NK, C, F), dtype=np.float32)
    in_maps = []
    for b in range(B):
        in_maps.append(
            {
                "x": np.ascontiguousarray(x[b].reshape(NPIX, C), np.float32),
                "off": np.ascontiguousarray(
                    offset[b].reshape(NPIX, 2 * NK), np.float32
                ),
                "mod": np.ascontiguousarray(
                    modulation[b].reshape(NPIX, NK), np.float32
                ),
                "ck": ck9,
                "bcat": consts["bcat"],
                "mask": consts["mask"],
                "identf": consts["identf"],
            }
        )
    nc = _get_nc()
    res = run_bass_kernel_spmd(
        nc,
        in_maps,
        core_ids=list(range(B)),
        trace=bool(int(os.environ.get("KERNEL_TRACE", "0"))),
    )
    outs = [res.results[b]["out"].reshape(H, W, F) for b in range(B)]
    result = np.stack(outs, axis=0) + conv_bias[None, None, None, :]
    if getattr(res, "exec_time_ns", None):
        kernel.last_exec_time_ns = res.exec_time_ns
    return result.astype(np.float32)
